# revision 34
# baseline (speedup 1.0000x reference)
"""Causal self-attention (B=2, T=2048, C=768, H=12) on 8 Trainium2 cores.

Sharding: 24 (batch, head) pairs / 8 cores = 3 heads per core.
core c -> batch b = c // 4, heads [3g, 3g+3) with g = c % 4.

Per-core device program (identical SPMD program, different input data):
  qkT  = (Wqk_local^T @ x_b^T)          [384, T]   (q cols pre-scaled 1/8,
                                                    q bias added, k bias
                                                    dropped: softmax-invariant)
  V    = x_b @ Wv_local                  [T, 192]   (v bias folded on host)
  per head h:
    scoresT[k, q] = kT_h^T-block @ qT_h  (PE, K=64; diagonal blocks trimmed
                                          to the causally-needed q columns)
    expT = exp(scoresT)                  (ACT; diagonal blocks multiplied
                                          by precomputed 0/1 masks on DVE,
                                          trimmed cols zero-filled)
    y_augT[[d;1], q] += V_aug^T @ expT   (PE, ones row -> softmax denom)
    yT_h = y_augT[y rows] * (1/denom)    (DVE approx-reciprocal; denom
                                          broadcast via gpsimd
                                          partition_broadcast)
  out_partial = Y_local @ Wp_local       [T, 768]   (emitted one q-chunk
                                          late to avoid PE head-of-line
                                          blocking on the yT writes)

Host: out[b] = sum of the 4 partials + (b_proj + b_v @ W_proj).

Matmuls run in bfloat16 (inputs converted host-side; PSUM accumulation
stays fp32). Halves HBM traffic and avoids the fp32r small-free-dim
matmul penalty; measured fro-rel error stays well inside the 2e-2 gate.

qkT feature-chunk layout (matmul needs lhsT/rhs on the same base
partition, so each head's q and k live at the same partition offset):
  chunk0 = [q0 | q2], chunk1 = [k0 | k2], chunk2 = [q1], chunk3 = [k1]
yT layout [128, 2, T]: h0 -> (0:64, 0), h1 -> (64:128, 0), h2 -> (0:64, 1)
so the out-projection fuses h0+h1 into one K=128 matmul.
V_aug per-kb free layout [65 | 128 | 65]:
  h0: [v_h0, 1]; h1: [1, 0*63, v_h1] (y rows 64:128, denom row 0);
  h2: [v_h2, 1]
"""

import numpy as np

import concourse.bass as bass
import concourse.mybir as mybir
import concourse.tile as tile
from concourse import bacc
from concourse import bass_utils

P = 128
D = 64          # head dim
HPC = 3         # heads per core
C = 768
CK = C // P     # 6 contraction chunks
QK = 2 * HPC * D  # 384 (q+k cols per core)
NH = 12
B = 2
N_CORES = 8
F32 = mybir.dt.float32
MM_DT = mybir.dt.bfloat16
F8 = mybir.dt.float8e4
NP_MM = mybir.dt.np(MM_DT)
DR = mybir.MatmulPerfMode.DoubleRow
CH = 96         # projection chain width (3 heads x 32 features)
NEG = -240.0    # additive causal mask; exp(0.125 * -240) = e^-30 ~ 0

# V_aug free-layout per head: (lhsT start, lhsT width, denom row, y row0).
# The ones column at 64 is shared between h0 (denom row 64) and h1 (row 0).
V_SLICE = [(0, 65, 64, 0), (64, 128, 0, 64), (192, 65, 64, 0)]
VW = 272
# yT destination (row0, chunk) per head
Y_POS = [(0, 0), (64, 0), (0, 1)]


def build_nc(T=2048, QCW=512):
    """Build the per-core Bass program. T = sequence length, QCW = q-chunk."""
    assert T % QCW == 0 and QCW % P == 0 and T % 512 == 0
    NQC = T // QCW
    NTB = T // P
    NPH = C // 2  # 384, out-proj free-dim half

    nc = bacc.Bacc("TRN2", target_bir_lowering=False, debug=False,
                   num_devices=N_CORES)
    xT = nc.dram_tensor("xT", [C, T], MM_DT, kind="ExternalInput").ap()
    wqk = nc.dram_tensor("wqk", [C, QK], MM_DT, kind="ExternalInput").ap()
    wv = nc.dram_tensor("wv", [C, HPC * D], MM_DT, kind="ExternalInput").ap()
    bqk = nc.dram_tensor("bqk", [512], F32, kind="ExternalInput").ap()
    wp = nc.dram_tensor("wp", [2 * P, C], MM_DT, kind="ExternalInput").ap()
    out = nc.dram_tensor("out", [T, C], MM_DT, kind="ExternalOutput").ap()

    Exp = mybir.ActivationFunctionType.Exp

    with tile.TileContext(nc) as tc:
        with (
            tc.tile_pool(name="const", bufs=1) as const,
            tc.tile_pool(name="work", bufs=8) as work,
            tc.tile_pool(name="small", bufs=3) as small,
            tc.tile_pool(name="outp", bufs=3) as outp,
            tc.tile_pool(name="ps_mm", bufs=2, space="PSUM") as ps_mm,
            tc.tile_pool(name="ps_s", bufs=4, space="PSUM") as ps_s,
            tc.tile_pool(name="ps_y", bufs=2, space="PSUM") as ps_y_pool,
        ):
            xT_sb = const.tile([P, CK, T], MM_DT, tag="xT")
            wqk_sb = const.tile([P, CK, QK], MM_DT, tag="wqk")
            wv_sb = const.tile([P, CK, HPC * D], MM_DT, tag="wv")
            bqk_sb = const.tile([P, 4], F32, tag="bqk")
            wp_sb = const.tile([P, 2, C], MM_DT, tag="wp")
            # packed q/k for fp8 DoubleRow scores: partitions 32h..32h+32
            # hold head h; free slots (0,1) = q d-lo/d-hi, (2,3) = k halves
            qk8 = const.tile([P, 4, T], F8, tag="qk8")
            v_sb = const.tile([P, NTB, VW], MM_DT, tag="v")
            yT_sb = const.tile([P, 2, T], MM_DT, tag="yT")
            zb_sb = const.tile([P, 1], F32, tag="zb")
            ones_sb = const.tile([1, P], MM_DT, tag="ones")
            mask_sb = const.tile([P, 4, QCW], F32, tag="mask")
            zq_sb = const.tile([P, 64], F32, tag="zq")

            # ---- loads (weights first; xT per (kc, tj) chunk, spread
            # across the sync/gpsimd/scalar DMA queues) ----
            dma_engs = [nc.sync, nc.gpsimd, nc.scalar]
            # per-kc (wqk, xT-slice0) pairs interleaved at the head of all
            # three queues so the first qkT matmul chain starts ~2us in,
            # instead of waiting behind a monolithic 1.15MB wqk transfer
            nc.scalar.dma_start(bqk_sb[:], bqk.rearrange("(ci p) -> p ci", p=P))
            for kc in range(CK):
                dma_engs[(2 * kc) % 3].dma_start(
                    wqk_sb[:, kc, :],
                    wqk[kc * P:(kc + 1) * P, :])
                dma_engs[(2 * kc + 1) % 3].dma_start(
                    xT_sb[:, kc, 0:512],
                    xT[kc * P:(kc + 1) * P, 0:512])
            nc.gpsimd.dma_start(wv_sb[:], wv.rearrange("(kc p) m -> p kc m", p=P))
            # xT slice 1 ahead of wp: wp is first needed by proj_qc(0),
            # long after project_tj(1) wants its xT chunks
            di = 0
            for kc in range(CK):
                dma_engs[di % 3].dma_start(
                    xT_sb[:, kc, 512:1024], xT[kc * P:(kc + 1) * P, 512:1024])
                di += 1
            nc.scalar.dma_start(wp_sb[:], wp.rearrange("(ci p) e -> p ci e", p=P))

            def issue_late_loads():
                # xT slices 2..3 are first needed by project_tj(2) ~40us
                # in; issuing them here (after project_tj(1) is emitted)
                # keeps the tiny tj<=1 k-hi relocation DMAs from queueing
                # behind ~6MB of prefetch in the same rings
                dj = di
                for tj in range(2, T // 512):
                    for kc in range(CK):
                        dma_engs[dj % 3].dma_start(
                            xT_sb[:, kc, tj * 512:(tj + 1) * 512],
                            xT[kc * P:(kc + 1) * P,
                               tj * 512:(tj + 1) * 512])
                        dj += 1

            # ---- constants (f32r tiles can't be memset; stage + cast) ----
            st = const.tile([P, 2], F32, tag="st")
            st1 = const.tile([1, P], F32, tag="st1")
            nc.gpsimd.memset(st[:], 1.0)
            nc.gpsimd.memset(st1[:], 1.0)
            nc.vector.tensor_copy(ones_sb[:], st1[:])
            nc.vector.tensor_copy(v_sb[:, :, 64:65],
                                  st[:, None, 0:1].to_broadcast((P, NTB, 1)))
            nc.vector.tensor_copy(v_sb[:, :, 256:257],
                                  st[:, None, 0:1].to_broadcast((P, NTB, 1)))
            nc.gpsimd.memset(zq_sb[:], 0.0)
            # h1 junk cols (feed only never-read psy rows); zero for sim
            nc.vector.tensor_copy(v_sb[:, :, 65:128],
                                  zq_sb[:, None, 0:63].to_broadcast((P, NTB, 63)))
            nc.gpsimd.memset(zb_sb[:], 0.0)
            # causal 0/1 masks: mask_j[x, y] = 1 if y - x >= 128*j else 0
            # additive causal masks: mask_j[x, y] = 0 if y - x >= 128*j
            # else NEG (applied to f32 scores in PSUM before exp)
            nc.gpsimd.memset(mask_sb[:], 0.0)
            for j in range(4):
                nc.gpsimd.affine_select(
                    mask_sb[:, j, :], mask_sb[:, j, :],
                    pattern=[[1, QCW]],
                    compare_op=mybir.AluOpType.is_ge,
                    fill=NEG,
                    base=-128 * j,
                    channel_multiplier=-1,
                )

            # ---- qkT + V projections for one 512-token slice.
            # chunks 2+3 ([q1|k1]) fused into one 128-row matmul chain; the
            # k1 half is relocated to chunk 3 partitions 0:64 by an
            # SBUF->SBUF DMA (engines can't cross partitions, DMA can). ----
            def project_tj(tj):
                tjs = slice(tj * 512, (tj + 1) * 512)
                # 4 chains of 96 features: q-lo, q-hi, k-lo, k-hi (per-head
                # 32-feature groups at partitions 32h) -> fp8 qk8 writes.
                # q bias added pre-cast; the 1/8 softmax scale is applied
                # by the exp activation instead of pre-scaling q.
                for c in range(3):
                    ps = ps_mm.tile([P, 512], F32, tag="mm")
                    for kc in range(CK):
                        nc.tensor.matmul(
                            ps[:, :],
                            wqk_sb[:, kc, c * P:(c + 1) * P],
                            xT_sb[:, kc, tjs],
                            start=(kc == 0), stop=(kc == CK - 1),
                        )
                    if c < 2:
                        nc.vector.tensor_scalar_add(
                            qk8[:, c, tjs], ps[:, :],
                            bqk_sb[:, c:c + 1])
                    else:
                        nc.vector.tensor_copy(qk8[:, 2, tjs], ps[:, :])
                    # k-hi of head c rides at partitions 96:128 of chain
                    # c (full-width M=128 chains); SBUF->SBUF DMA
                    # relocates it to (32c:32c+32, slot 3) for the score
                    # matmul's same-base-partition requirement
                    dma_engs[c % 3].dma_start(
                        qk8[32 * c:32 * c + 32, 3, tjs],
                        qk8[96:128, c, tjs])
                for tb in range(4 * tj, 4 * tj + 4):
                    ps = ps_mm.tile([P, 512], F32, tag="mm")
                    for kc in range(CK):
                        nc.tensor.matmul(
                            ps[:, :HPC * D],
                            xT_sb[:, kc, tb * P:(tb + 1) * P],
                            wv_sb[:, kc, :],
                            start=(kc == 0), stop=(kc == CK - 1),
                        )
                    nc.vector.tensor_copy(v_sb[:, tb, 0:64], ps[:, 0:64])
                    nc.vector.tensor_copy(v_sb[:, tb, 128:256],
                                          ps[:, 64:192])

            # ---- attention (per q-chunk) and delayed out-projection.
            # Normalization of head i is emitted during head i+1's matmul
            # loop so its ACT->DVE->PE chain never stalls the in-order PE.
            pend1, pend2 = [], []

            def norm_stage1(st8):
                qc, h, psy_t = st8
                v0, vw, srow, yrow = V_SLICE[h]
                den = small.tile([1, QCW], F32, tag="den")
                # DVE, not ACT: the copy would otherwise queue behind
                # ~2 pending exps and delay the recip chain that
                # stage2's PE broadcast waits on
                nc.vector.tensor_copy(den[:], psy_t[srow:srow + 1, :])
                recf = small.tile([1, QCW], F32, tag="recf")
                nc.vector.reciprocal_approx_fast(recf[:], den[:])
                recip = small.tile([1, QCW], MM_DT, tag="recip")
                nc.vector.tensor_copy(recip[:], recf[:])
                return (qc, h, psy_t, recip)

            def norm_stage2(st8):
                qc, h, psy_t, recip = st8
                q0 = qc * QCW
                v0, vw, srow, yrow = V_SLICE[h]
                psb = ps_mm.tile([P, QCW], F32, tag="mm", name="psb")
                nc.tensor.matmul(psb[:], ones_sb[:], recip[:],
                                 start=True, stop=True)
                bc = small.tile([P, QCW], F32, tag="bcs")
                yp, yci = Y_POS[h]
                nc.vector.tensor_copy(bc[yrow:yrow + D, :],
                                      psb[yrow:yrow + D, :])
                nc.vector.tensor_mul(
                    yT_sb[yp:yp + D, yci, q0:q0 + QCW],
                    psy_t[yrow:yrow + D, :], bc[yrow:yrow + D, :])

            def attn_qc(qc, fillers=()):
                fl = list(fillers)
                q0 = qc * QCW
                nkb = q0 // P + 4
                for h in range(HPC):
                    hp = 32 * h
                    v0, vw, srow, yrow = V_SLICE[h]
                    psy_t = ps_y_pool.tile([P, QCW], F32, tag="yaug",
                                           name="psy")
                    psy = psy_t[0:vw, :]
                    # AV matmuls trail their exp by 2 blocks so the
                    # in-order PE never waits on ACT: between score(kb)
                    # and AV(kb) the PE runs score(kb+1), AV(kb-1),
                    # score(kb+2) -- ~2.4us of cover for the ~1.9us
                    # score->mask->exp->AV dependency chain.
                    avq = []

                    def flush_av():
                        kb, n0, ex = avq.pop(0)
                        nc.tensor.matmul(
                            psy[:, n0:], v_sb[:, kb, v0:v0 + vw],
                            ex[:, n0:],
                            start=(kb == 0), stop=(kb == nkb - 1))

                    for kb in range(nkb):
                        # diagonal blocks only need q columns >= kb*128
                        n0 = max(0, kb * P - q0)
                        pss = ps_s.tile([P, QCW], F32, tag="ss")
                        nc.tensor.matmul(
                            pss[:, n0:],
                            qk8[hp:hp + 32, 2:4, kb * P:(kb + 1) * P],
                            qk8[hp:hp + 32, 0:2, q0 + n0:q0 + QCW],
                            start=True, stop=True, perf_mode=DR,
                        )
                        if kb * P >= q0:
                            # additive causal mask; block j differs from
                            # "keep everything" only in cols [n0, n0+128)
                            j = kb - q0 // P
                            nc.vector.tensor_add(
                                pss[:, n0:n0 + P], pss[:, n0:n0 + P],
                                mask_sb[:, j, n0:n0 + P])
                        ex = work.tile([P, QCW], MM_DT, tag="expT")
                        nc.scalar.activation(ex[:, n0:], pss[:, n0:], Exp,
                                             bias=zb_sb[:], scale=0.125)
                        avq.append((kb, n0, ex))
                        # trail depth 4: steady cadence stays S,AV,S,AV
                        # but head starts get more back-to-back scores
                        # and head ends more back-to-back AVs
                        # (same-kind neighbors overlap LDWEIGHTS)
                        if len(avq) > 4:
                            flush_av()
                        if kb >= 3 and kb % 2 == 1 and pend2:
                            # odd kb: one extra block of slack for the
                            # prior head's recip chain to clear the DVE
                            # queue before stage2's PE broadcast needs it
                            norm_stage2(pend2.pop(0))
                    while avq:
                        flush_av()
                    # den/recip right after this head's last AV: the den
                    # copy lands in ACT's FIFO ahead of the next head's
                    # exps, so recip is long done when stage2's PE
                    # broadcast fires a head later.
                    pend2.append(norm_stage1((qc, h, psy_t)))
                    # out-projection filler: keeps the PE fed while the
                    # next head's score->exp chain drains on ACT
                    if h < 2:
                        for _ in range(2):
                            if fl:
                                fl.pop(0)()
                while fl:
                    fl.pop(0)()

            def proj_tb(tb):
                osb = outp.tile([P, C], MM_DT, tag="osb")
                # both halves share the same two stationaries (the yT
                # token block's chunks); emit chunk-major so identical
                # LDWEIGHTS are back-to-back instead of alternating
                psos = [ps_mm.tile([P, 512], F32, tag="mm",
                                   name="pso")[:, :NPH]
                        for _ in range(2)]
                for half in range(2):
                    nc.tensor.matmul(
                        psos[half], yT_sb[:, 0, tb * P:(tb + 1) * P],
                        wp_sb[:, 0, half * NPH:(half + 1) * NPH],
                        start=True, stop=False)
                for half in range(2):
                    nc.tensor.matmul(
                        psos[half], yT_sb[0:D, 1, tb * P:(tb + 1) * P],
                        wp_sb[0:D, 1, half * NPH:(half + 1) * NPH],
                        start=False, stop=True)
                for half in range(2):
                    nc.vector.tensor_copy(
                        osb[:, half * NPH:(half + 1) * NPH], psos[half])
                dma_engs[tb % 3].dma_start(
                    out[tb * P:(tb + 1) * P, :], osb[:])

            def proj_qc(qc):
                q0 = qc * QCW
                for tb in range(q0 // P, (q0 + QCW) // P):
                    proj_tb(tb)

            # interleaved pipeline: projections(tj) -> attention(tj),
            # with out-projection(tj-1) token blocks woven between
            # attention heads as PE filler (all three tj-1 norm stage2s
            # are popped during head 0's block loop, so yT(tj-1) is
            # complete by the first filler)
            for tj in range(T // 512):
                project_tj(tj)
                if tj == 1:
                    issue_late_loads()
                if tj > 0:
                    q0p = (tj - 1) * QCW
                    fillers = [(lambda tb=tb: proj_tb(tb))
                               for tb in range(q0p // P, q0p // P + 4)]
                else:
                    fillers = []
                attn_qc(tj, fillers)
            while pend2:
                norm_stage2(pend2.pop(0))
            proj_qc(NQC - 1)


    nc.compile()
    return nc


_NC_CACHE = {}


def _get_nc(T=2048, QCW=512):
    key = (T, QCW)
    if key not in _NC_CACHE:
        _NC_CACHE[key] = build_nc(T, QCW)
    return _NC_CACHE[key]


def build_in_maps(inputs):
    """Build the 8 per-core input dicts from full inputs."""
    x = np.asarray(inputs["x"], np.float32)
    W = np.asarray(inputs["W_attn"], np.float32)
    b = np.asarray(inputs["b_attn"], np.float32)
    W_proj = np.asarray(inputs["W_proj"], np.float32)
    in_maps = []
    for c in range(N_CORES):
        bi, g = divmod(c, 4)
        lo = g * (HPC * D)  # local head col offset within each of q/k/v
        # 4 chains of 96: q-lo, q-hi, k-lo, k-hi; within a chain, head-
        # major 32-feature groups. No 1/8 pre-scale on q (the exp
        # activation applies it), so q/k stay in fp8's sweet spot.
        qw = W[:, lo:lo + HPC * D].reshape(C, HPC, D)
        kw = W[:, C + lo:C + lo + HPC * D].reshape(C, HPC, D)
        # 3 full-width chains of 128: [q-lo(3 heads) | k-hi h0],
        # [q-hi | k-hi h1], [k-lo | k-hi h2]; the k-hi strips are
        # relocated to qk8 slot 3 on-device by an SBUF->SBUF DMA
        wqk = np.concatenate([
            qw[:, :, 0:32].reshape(C, CH), kw[:, 0, 32:64],
            qw[:, :, 32:64].reshape(C, CH), kw[:, 1, 32:64],
            kw[:, :, 0:32].reshape(C, CH), kw[:, 2, 32:64],
        ], axis=1)
        qb = b[lo:lo + HPC * D].reshape(HPC, D)
        z32 = np.zeros(32, np.float32)
        bqk = np.concatenate([
            qb[:, 0:32].reshape(CH), z32, qb[:, 32:64].reshape(CH), z32,
            np.zeros(2 * P, np.float32)])
        wv = W[:, 2 * C + lo:2 * C + lo + HPC * D]
        # wp rows: [h0 | h1 | h2 | zero pad] -> chunks (0:128), (128:256)
        wp = np.zeros((2 * P, C), np.float32)
        wp[:HPC * D] = W_proj[lo:lo + HPC * D]
        in_maps.append({
            "xT": np.ascontiguousarray(x[bi].T).astype(NP_MM),
            "wqk": np.ascontiguousarray(wqk).astype(NP_MM),
            "wv": np.ascontiguousarray(wv).astype(NP_MM),
            "bqk": np.ascontiguousarray(bqk),
            "wp": np.ascontiguousarray(wp).astype(NP_MM),
        })
    return in_maps


def postprocess(results, inputs):
    b_attn = np.asarray(inputs["b_attn"], np.float32)
    W_proj = np.asarray(inputs["W_proj"], np.float32)
    b_proj = np.asarray(inputs["b_proj"], np.float32)
    b_eff = b_proj + b_attn[2 * C:] @ W_proj
    T = results[0]["out"].shape[0]
    out = np.zeros((B, T, C), np.float32)
    for c in range(N_CORES):
        out[c // 4] += results[c]["out"].astype(np.float32)
    out += b_eff
    return out


def kernel(x, W_attn, b_attn, W_proj, b_proj):
    inputs = dict(x=x, W_attn=W_attn, b_attn=b_attn,
                  W_proj=W_proj, b_proj=b_proj)
    T = np.asarray(x).shape[1]
    nc = _get_nc(T=T)
    in_maps = build_in_maps(inputs)
    res = bass_utils.run_bass_kernel_spmd(
        nc, in_maps, core_ids=list(range(N_CORES)))
    return postprocess(res.results, inputs)

